# revision 10
# baseline (speedup 1.0000x reference)
"""Trainium2 Bass kernel for nn_LocalPODLoss.

Reference computation:
  D = new_f - old_f,  shape [B=16, C=512, W=32, H=32]
  With S=2 scales only the s=1 (16x16 window) scale contributes:
    ss = (1/256) * sum_img [ sum_{k in 0..15, h} m(h) * ROW[k,h]^2
                           + sum_{w, k in 0..15} m(w) * COL[w,k]^2 ]
    ROW[k,h] = sum_{r=k..k+15} D[r,h]   (windowed sums along W)
    COL[w,k] = sum_{t=k..k+15} D[w,t]   (windowed sums along H)
    m(x) = min(x+1, 31-x) window-multiplicity weight (m(31)=0)
  out = 0.5 * (1e-6 + sqrt(ss))

Kernel strategy (8 NeuronCores, data-parallel over batch):
  Each core handles 2 batches = 1024 images of 32x32, cast to bf16 on the
  host (halves HBM traffic; rounding error ~1e-4 on the final scalar).
  SBUF layout per 128-image chunk: X[(g,w), (G,h)] = img(g,G)[w,h] with
  g in 0..3, G in 0..31 (host pre-interleaves), so the PE matmul with a
  block-diagonal banded moving matrix computes per-image window sums:
    out_L[(G4,h), (g,k)] = sum_w band[w,k] * D_img[w,h]   (row sums)
  placing the weight axis (h resp. w) on PSUM partitions.
  - D = new - old on DVE (even chunks) / GpSimd (odd chunks); the
    32x32 block transposes for the column path stay on DVE (the
    single-port stream transpose is immune to the GpSimd port share).
  - PE: data as stationary (bf16 LDWEIGHTS pipelines against the moving
    pass via the dual weight banks), banded matrix moving.
  - One ACT pass per chunk PAIR over a 4-bank PSUM tile:
    activation(Square, scale=sqrt(m(p%32))/16, accum_out) handles both
    row and col terms (same weight pattern mod 32).
  Per-core partial sums [128, 4] are DMA'd out; the host sums them,
  adds eps, takes sqrt.
"""

import numpy as np

B, C, W, H = 16, 512, 32, 32
NCORES = 8
IMGS_PER_CORE = (B // NCORES) * C          # 1024
NCHUNK = 8                                  # chunks per core
NPAIR = NCHUNK // 2
FREE = 1024                                 # (G, h) free elements per chunk

_cache = {}


def _consts():
    # m(x) multiplicity weights; m(31) = 0
    m = np.minimum(np.arange(32) + 1, 31 - np.arange(32)).astype(np.float64)
    m[31] = 0.0
    # per-partition scale s[p] = sqrt(m(p%32))/16  (so s^2 = m/256)
    svec = (np.sqrt(np.tile(m, 4)) / 16.0).astype(np.float32).reshape(128, 1)
    # block-diagonal banded moving matrix [128, 64]:
    # MBLK[(a,x), (b,k)] = (a==b) * (k <= x < k+16)
    mblk = np.zeros((128, 64), dtype=np.float32)
    for a in range(4):
        for x in range(32):
            for k in range(16):
                if k <= x < k + 16:
                    mblk[a * 32 + x, a * 16 + k] = 1.0
    return mblk, svec


def _build():
    if "nc" in _cache:
        return _cache["nc"]

    import concourse.bacc as bacc
    import concourse.tile as tile
    from concourse import mybir

    f32 = mybir.dt.float32
    bf16 = mybir.dt.bfloat16
    nc = bacc.Bacc("TRN2", target_bir_lowering=False, debug=False,
                   num_devices=NCORES)

    # host-prearranged: row ch*128 + g*32 + w, col G*32 + h = img(g,G)[w,h]
    # of chunk ch; new in cols 0:1024, old in cols 1024:2048.
    big = nc.dram_tensor("big", [NCHUNK * 128, 2 * FREE], bf16,
                         kind="ExternalInput")
    mblk_d = nc.dram_tensor("mblk", [128, 64], bf16, kind="ExternalInput")
    svec_d = nc.dram_tensor("svec", [128, 1], f32, kind="ExternalInput")
    partials = nc.dram_tensor("partials", [128, NPAIR], f32,
                              kind="ExternalOutput")

    big_v = big.ap().rearrange("(c p) f -> c p f", p=128)

    class _FastTileContext(tile.TileContext):
        """Same as TileContext but exits with sem-only barriers: skips the
        end-of-kernel drain butterfly (GpSimd dge_drains are ~0.5-2us each;
        the semaphores are re-initialised by the next run's preamble)."""

        def _drain_and_barrier(self, tick_clock, wait_clock):
            from concourse.vector_clock import ScopedClock
            drain_inst = self.nc.sync.drain()
            wait_clock.add_sem_waits(
                drain_inst.ins, ScopedClock({None: tick_clock.global_clock})
            )
            self.nc.all_engine_barrier(sem_only=True)
            popped = self.nc._tile_sem_poison_stack.pop()
            assert popped is self._sem_poison
            self.nc.clear_and_free_semaphores(
                list(self.sems.allocated().values()))
            self.nc.all_engine_barrier(sem_only=True)

    with _FastTileContext(nc) as tc:
        with (
            tc.tile_pool(name="consts", bufs=1) as consts,
            tc.tile_pool(name="loads", bufs=6) as loads,
            tc.tile_pool(name="work", bufs=4) as work,
            tc.tile_pool(name="sq", bufs=2) as sqp,
            tc.tile_pool(name="acc", bufs=1) as accp,
            tc.tile_pool(name="psum", bufs=2, space="PSUM") as psum,
        ):
            mblk_t = consts.tile([128, 64], bf16)
            nc.scalar.dma_start(mblk_t[:], mblk_d.ap())
            svec_t = consts.tile([128, 1], f32)
            nc.scalar.dma_start(svec_t[:], svec_d.ap())
            acc = accp.tile([128, NPAIR], f32)

            for p in range(NPAIR):
                # 4-bank PSUM tile per chunk pair; chunk 2p in cols 0:1024,
                # chunk 2p+1 in cols 1024:2048
                ps = psum.tile([128, 2 * FREE], f32)
                for half in range(2):
                    c = 2 * p + half
                    pair = loads.tile([128, 2 * FREE], bf16)
                    if c < 2:
                        # split the pipeline-head loads across both HWDGE
                        # queues so the first sub starts sooner
                        nc.sync.dma_start(pair[:, :FREE], big_v[c][:, :FREE])
                        nc.scalar.dma_start(pair[:, FREE:], big_v[c][:, FREE:])
                    else:
                        nc.sync.dma_start(pair[:], big_v[c])

                    d_t = work.tile([128, FREE], bf16)
                    # GpSimd takes the middle chunks; the DVE's own subs sit
                    # at the ends where GpSimd is idle (the shared SBUF port
                    # otherwise stalls DVE tensor_tensor ops ~3x)
                    if c in (1, 2, 3, 4, 5):
                        nc.gpsimd.tensor_sub(d_t[:], pair[:, :FREE],
                                             pair[:, FREE:])
                    else:
                        nc.vector.tensor_sub(d_t[:], pair[:, :FREE],
                                             pair[:, FREE:])
                    dt_t = work.tile([128, FREE], bf16)
                    nc.vector.transpose(dt_t[:], d_t[:])

                    off = half * FREE
                    for j in range(FREE // 128):
                        nc.tensor.matmul(
                            ps[:, off + j * 64:off + (j + 1) * 64],
                            d_t[:, j * 128:(j + 1) * 128],
                            mblk_t[:],
                            start=True, stop=True,
                        )
                    for j in range(FREE // 128):
                        nc.tensor.matmul(
                            ps[:, off + 512 + j * 64:off + 512 + (j + 1) * 64],
                            dt_t[:, j * 128:(j + 1) * 128],
                            mblk_t[:],
                            start=True, stop=True,
                        )

                # both chunks + both terms in one ACT pass:
                # accum_out = sum over free of (svec*ps)^2
                sq = sqp.tile([128, 2 * FREE], bf16)
                nc.scalar.activation(
                    sq[:], ps[:], mybir.ActivationFunctionType.Square,
                    scale=svec_t[:], accum_out=acc[:, p:p + 1],
                )

            nc.sync.dma_start(partials.ap(), acc[:])

    nc.compile()
    _cache["nc"] = nc
    return nc


def _prep_core(arr_bf, k):
    """arr_bf: full [16, 512, 32, 32] array; returns [1024, 1024]
    chunk-major layout for core k: [ch, g, w, G, h]."""
    bpc = B // NCORES
    imgs = arr_bf[k * bpc:(k + 1) * bpc].reshape(NCHUNK, 4, 32, W, H)
    return np.ascontiguousarray(
        imgs.transpose(0, 1, 3, 2, 4)).reshape(NCHUNK * 128, FREE)


def _run(new_f, old_f, trace=False, **trace_kwargs):
    import ml_dtypes
    from concourse.bass_utils import run_bass_kernel_spmd

    nc = _build()
    mblk, svec = _consts()
    mblk_bf = mblk.astype(ml_dtypes.bfloat16)
    new_bf = np.asarray(new_f, dtype=ml_dtypes.bfloat16)
    old_bf = np.asarray(old_f, dtype=ml_dtypes.bfloat16)
    in_maps = []
    for k in range(NCORES):
        bigk = np.empty((NCHUNK * 128, 2 * FREE), dtype=ml_dtypes.bfloat16)
        bigk[:, :FREE] = _prep_core(new_bf, k)
        bigk[:, FREE:] = _prep_core(old_bf, k)
        in_maps.append({
            "big": bigk,
            "mblk": mblk_bf,
            "svec": svec,
        })
    res = run_bass_kernel_spmd(nc, in_maps, list(range(NCORES)),
                               trace=trace, **trace_kwargs)
    ss = np.float64(0.0)
    for k in range(NCORES):
        ss += np.float64(res.results[k]["partials"].astype(np.float64).sum())
    out = np.float32(0.5 * (np.float32(1e-6) + np.float32(np.sqrt(np.float32(ss)))))
    return np.asarray(out, dtype=np.float32), res


def kernel(new_f, old_f):
    out, _ = _run(np.asarray(new_f), np.asarray(old_f))
    return out


# revision 11
# speedup vs baseline: 1.0832x; 1.0832x over previous
"""Trainium2 Bass kernel for nn_LocalPODLoss.

Reference computation:
  D = new_f - old_f,  shape [B=16, C=512, W=32, H=32]
  With S=2 scales only the s=1 (16x16 window) scale contributes:
    ss = (1/256) * sum_img [ sum_{k in 0..15, h} m(h) * ROW[k,h]^2
                           + sum_{w, k in 0..15} m(w) * COL[w,k]^2 ]
    ROW[k,h] = sum_{r=k..k+15} D[r,h]   (windowed sums along W)
    COL[w,k] = sum_{t=k..k+15} D[w,t]   (windowed sums along H)
    m(x) = min(x+1, 31-x) window-multiplicity weight (m(31)=0)
  out = 0.5 * (1e-6 + sqrt(ss))

Kernel strategy (8 NeuronCores, data-parallel over batch):
  Each core handles 2 batches = 1024 images of 32x32, cast to bf16 on the
  host (halves HBM traffic; rounding error ~1e-4 on the final scalar).
  SBUF layout per 128-image chunk: X[(g,w), (G,h)] = img(g,G)[w,h] with
  g in 0..3, G in 0..31 (host pre-interleaves), so the PE matmul with a
  block-diagonal banded moving matrix computes per-image window sums:
    out_L[(G4,h), (g,k)] = sum_w band[w,k] * D_img[w,h]   (row sums)
  placing the weight axis (h resp. w) on PSUM partitions.
  - D = new - old on DVE (even chunks) / GpSimd (odd chunks); the
    32x32 block transposes for the column path stay on DVE (the
    single-port stream transpose is immune to the GpSimd port share).
  - PE: data as stationary (bf16 LDWEIGHTS pipelines against the moving
    pass via the dual weight banks), banded matrix moving.
  - One ACT pass per chunk PAIR over a 4-bank PSUM tile:
    activation(Square, scale=sqrt(m(p%32))/16, accum_out) handles both
    row and col terms (same weight pattern mod 32).
  Per-core partial sums [128, 4] are DMA'd out; the host sums them,
  adds eps, takes sqrt.
"""

import numpy as np

B, C, W, H = 16, 512, 32, 32
NCORES = 8
IMGS_PER_CORE = (B // NCORES) * C          # 1024
NCHUNK = 8                                  # chunks per core
NPAIR = NCHUNK // 2
FREE = 1024                                 # (G, h) free elements per chunk

_cache = {}


def _consts():
    # m(x) multiplicity weights; m(31) = 0
    m = np.minimum(np.arange(32) + 1, 31 - np.arange(32)).astype(np.float64)
    m[31] = 0.0
    # per-partition scale s[p] = sqrt(m(p%32))/16  (so s^2 = m/256)
    svec = (np.sqrt(np.tile(m, 4)) / 16.0).astype(np.float32).reshape(128, 1)
    # block-diagonal banded moving matrix [128, 64]:
    # MBLK[(a,x), (b,k)] = (a==b) * (k <= x < k+16)
    mblk = np.zeros((128, 64), dtype=np.float32)
    for a in range(4):
        for x in range(32):
            for k in range(16):
                if k <= x < k + 16:
                    mblk[a * 32 + x, a * 16 + k] = 1.0
    return mblk, svec


def _build():
    if "nc" in _cache:
        return _cache["nc"]

    import concourse.bacc as bacc
    import concourse.tile as tile
    from concourse import mybir

    f32 = mybir.dt.float32
    bf16 = mybir.dt.bfloat16
    nc = bacc.Bacc("TRN2", target_bir_lowering=False, debug=False,
                   num_devices=NCORES)

    # host-prearranged: row ch*128 + g*32 + w, col G*32 + h = img(g,G)[w,h]
    # of chunk ch; new in cols 0:1024, old in cols 1024:2048.
    big = nc.dram_tensor("big", [NCHUNK * 128, 2 * FREE], bf16,
                         kind="ExternalInput")
    mblk_d = nc.dram_tensor("mblk", [128, 64], bf16, kind="ExternalInput")
    svec_d = nc.dram_tensor("svec", [128, 1], f32, kind="ExternalInput")
    partials = nc.dram_tensor("partials", [128, NPAIR], f32,
                              kind="ExternalOutput")

    big_v = big.ap().rearrange("(c p) f -> c p f", p=128)

    class _FastTileContext(tile.TileContext):
        """Same as TileContext but exits with sem-only barriers: skips the
        end-of-kernel drain butterfly (GpSimd dge_drains are ~0.5-2us each;
        the semaphores are re-initialised by the next run's preamble)."""

        def _drain_and_barrier(self, tick_clock, wait_clock):
            from concourse.vector_clock import ScopedClock
            drain_inst = self.nc.sync.drain()
            wait_clock.add_sem_waits(
                drain_inst.ins, ScopedClock({None: tick_clock.global_clock})
            )
            self.nc.all_engine_barrier(sem_only=True)
            popped = self.nc._tile_sem_poison_stack.pop()
            assert popped is self._sem_poison
            self.nc.clear_and_free_semaphores(
                list(self.sems.allocated().values()))
            self.nc.all_engine_barrier(sem_only=True)

    with _FastTileContext(nc) as tc:
        with (
            tc.tile_pool(name="consts", bufs=1) as consts,
            tc.tile_pool(name="loads", bufs=6) as loads,
            tc.tile_pool(name="work", bufs=4) as work,
            tc.tile_pool(name="sq", bufs=2) as sqp,
            tc.tile_pool(name="acc", bufs=1) as accp,
            tc.tile_pool(name="psum", bufs=2, space="PSUM") as psum,
        ):
            mblk_t = consts.tile([128, 64], bf16)
            nc.scalar.dma_start(mblk_t[:], mblk_d.ap())
            svec_t = consts.tile([128, 1], f32)
            nc.scalar.dma_start(svec_t[:], svec_d.ap())
            acc = accp.tile([128, NPAIR], f32)

            for p in range(NPAIR):
                # 4-bank PSUM tile per chunk pair; chunk 2p in cols 0:1024,
                # chunk 2p+1 in cols 1024:2048
                ps = psum.tile([128, 2 * FREE], f32)
                for half in range(2):
                    c = 2 * p + half
                    pair = loads.tile([128, 2 * FREE], bf16)
                    if c < 2:
                        # split the pipeline-head loads across both HWDGE
                        # queues so the first sub starts sooner
                        nc.sync.dma_start(pair[:, :FREE], big_v[c][:, :FREE])
                        nc.scalar.dma_start(pair[:, FREE:], big_v[c][:, FREE:])
                    else:
                        nc.sync.dma_start(pair[:], big_v[c])

                    d_t = work.tile([128, FREE], bf16)
                    if c % 2 == 1:
                        nc.gpsimd.tensor_sub(d_t[:], pair[:, :FREE],
                                             pair[:, FREE:])
                    else:
                        nc.vector.tensor_sub(d_t[:], pair[:, :FREE],
                                             pair[:, FREE:])
                    dt_t = work.tile([128, FREE], bf16)
                    nc.vector.transpose(dt_t[:], d_t[:])

                    off = half * FREE
                    for j in range(FREE // 128):
                        nc.tensor.matmul(
                            ps[:, off + j * 64:off + (j + 1) * 64],
                            d_t[:, j * 128:(j + 1) * 128],
                            mblk_t[:],
                            start=True, stop=True,
                        )
                    for j in range(FREE // 128):
                        nc.tensor.matmul(
                            ps[:, off + 512 + j * 64:off + 512 + (j + 1) * 64],
                            dt_t[:, j * 128:(j + 1) * 128],
                            mblk_t[:],
                            start=True, stop=True,
                        )

                # both chunks + both terms in one ACT pass:
                # accum_out = sum over free of (svec*ps)^2
                sq = sqp.tile([128, 2 * FREE], bf16)
                nc.scalar.activation(
                    sq[:], ps[:], mybir.ActivationFunctionType.Square,
                    scale=svec_t[:], accum_out=acc[:, p:p + 1],
                )

            nc.sync.dma_start(partials.ap(), acc[:])

    nc.compile()
    _cache["nc"] = nc
    return nc


def _prep_core(arr_bf, k):
    """arr_bf: full [16, 512, 32, 32] array; returns [1024, 1024]
    chunk-major layout for core k: [ch, g, w, G, h]."""
    bpc = B // NCORES
    imgs = arr_bf[k * bpc:(k + 1) * bpc].reshape(NCHUNK, 4, 32, W, H)
    return np.ascontiguousarray(
        imgs.transpose(0, 1, 3, 2, 4)).reshape(NCHUNK * 128, FREE)


def _run(new_f, old_f, trace=False, **trace_kwargs):
    import ml_dtypes
    from concourse.bass_utils import run_bass_kernel_spmd

    nc = _build()
    mblk, svec = _consts()
    mblk_bf = mblk.astype(ml_dtypes.bfloat16)
    new_bf = np.asarray(new_f, dtype=ml_dtypes.bfloat16)
    old_bf = np.asarray(old_f, dtype=ml_dtypes.bfloat16)
    in_maps = []
    for k in range(NCORES):
        bigk = np.empty((NCHUNK * 128, 2 * FREE), dtype=ml_dtypes.bfloat16)
        bigk[:, :FREE] = _prep_core(new_bf, k)
        bigk[:, FREE:] = _prep_core(old_bf, k)
        in_maps.append({
            "big": bigk,
            "mblk": mblk_bf,
            "svec": svec,
        })
    res = run_bass_kernel_spmd(nc, in_maps, list(range(NCORES)),
                               trace=trace, **trace_kwargs)
    ss = np.float64(0.0)
    for k in range(NCORES):
        ss += np.float64(res.results[k]["partials"].astype(np.float64).sum())
    out = np.float32(0.5 * (np.float32(1e-6) + np.float32(np.sqrt(np.float32(ss)))))
    return np.asarray(out, dtype=np.float32), res


def kernel(new_f, old_f):
    out, _ = _run(np.asarray(new_f), np.asarray(old_f))
    return out


# revision 12
# speedup vs baseline: 1.1760x; 1.0857x over previous
"""Trainium2 Bass kernel for nn_LocalPODLoss.

Reference computation:
  D = new_f - old_f,  shape [B=16, C=512, W=32, H=32]
  With S=2 scales only the s=1 (16x16 window) scale contributes:
    ss = (1/256) * sum_img [ sum_{k in 0..15, h} m(h) * ROW[k,h]^2
                           + sum_{w, k in 0..15} m(w) * COL[w,k]^2 ]
    ROW[k,h] = sum_{r=k..k+15} D[r,h]   (windowed sums along W)
    COL[w,k] = sum_{t=k..k+15} D[w,t]   (windowed sums along H)
    m(x) = min(x+1, 31-x) window-multiplicity weight (m(31)=0)
  out = 0.5 * (1e-6 + sqrt(ss))

Kernel strategy (8 NeuronCores, data-parallel over batch):
  Each core handles 2 batches = 1024 images of 32x32, cast to bf16 on the
  host (halves HBM traffic; rounding error ~1e-4 on the final scalar).
  SBUF layout per 128-image chunk: X[(g,w), (G,h)] = img(g,G)[w,h] with
  g in 0..3, G in 0..31 (host pre-interleaves), so the PE matmul with a
  block-diagonal banded moving matrix computes per-image window sums:
    out_L[(G4,h), (g,k)] = sum_w band[w,k] * D_img[w,h]   (row sums)
  placing the weight axis (h resp. w) on PSUM partitions.
  - D = new - old on DVE (even chunks) / GpSimd (odd chunks); the
    32x32 block transposes for the column path stay on DVE (the
    single-port stream transpose is immune to the GpSimd port share).
  - PE: data as stationary (bf16 LDWEIGHTS pipelines against the moving
    pass via the dual weight banks), banded matrix moving.
  - One ACT pass per chunk PAIR over a 4-bank PSUM tile:
    activation(Square, scale=sqrt(m(p%32))/16, accum_out) handles both
    row and col terms (same weight pattern mod 32).
  Per-core partial sums [128, 4] are DMA'd out; the host sums them,
  adds eps, takes sqrt.
"""

import numpy as np

B, C, W, H = 16, 512, 32, 32
NCORES = 8
IMGS_PER_CORE = (B // NCORES) * C          # 1024
NCHUNK = 8                                  # chunks per core
NPAIR = NCHUNK // 2
FREE = 1024                                 # (G, h) free elements per chunk

_cache = {}


def _consts():
    # m(x) multiplicity weights; m(31) = 0
    m = np.minimum(np.arange(32) + 1, 31 - np.arange(32)).astype(np.float64)
    m[31] = 0.0
    # per-partition scale s[p] = sqrt(m(p%32))/16  (so s^2 = m/256)
    svec = (np.sqrt(np.tile(m, 4)) / 16.0).astype(np.float32).reshape(128, 1)
    # block-diagonal banded moving matrix [128, 64]:
    # MBLK[(a,x), (b,k)] = (a==b) * (k <= x < k+16)
    mblk = np.zeros((128, 64), dtype=np.float32)
    for a in range(4):
        for x in range(32):
            for k in range(16):
                if k <= x < k + 16:
                    mblk[a * 32 + x, a * 16 + k] = 1.0
    return mblk, svec


def _build():
    if "nc" in _cache:
        return _cache["nc"]

    import concourse.bacc as bacc
    import concourse.tile as tile
    from concourse import mybir

    f32 = mybir.dt.float32
    bf16 = mybir.dt.bfloat16
    nc = bacc.Bacc("TRN2", target_bir_lowering=False, debug=False,
                   num_devices=NCORES)

    # host-prearranged: row ch*128 + g*32 + w, col G*32 + h = img(g,G)[w,h]
    # of chunk ch; new in cols 0:1024, old in cols 1024:2048.
    big = nc.dram_tensor("big", [NCHUNK * 128, 2 * FREE], bf16,
                         kind="ExternalInput")
    mblk_d = nc.dram_tensor("mblk", [128, 64], bf16, kind="ExternalInput")
    svec_d = nc.dram_tensor("svec", [128, 1], f32, kind="ExternalInput")
    partials = nc.dram_tensor("partials", [128, NPAIR], f32,
                              kind="ExternalOutput")

    big_v = big.ap().rearrange("(c p) f -> c p f", p=128)

    class _FastTileContext(tile.TileContext):
        """Same as TileContext but exits with sem-only barriers: skips the
        end-of-kernel drain butterfly (GpSimd dge_drains are ~0.5-2us each;
        the semaphores are re-initialised by the next run's preamble)."""

        def _drain_and_barrier(self, tick_clock, wait_clock):
            from concourse.vector_clock import ScopedClock
            drain_inst = self.nc.sync.drain()
            wait_clock.add_sem_waits(
                drain_inst.ins, ScopedClock({None: tick_clock.global_clock})
            )
            self.nc.all_engine_barrier(sem_only=True)
            popped = self.nc._tile_sem_poison_stack.pop()
            assert popped is self._sem_poison
            self.nc.clear_and_free_semaphores(
                list(self.sems.allocated().values()))
            self.nc.all_engine_barrier(sem_only=True)

    with _FastTileContext(nc) as tc:
        with (
            tc.tile_pool(name="consts", bufs=1) as consts,
            tc.tile_pool(name="loads", bufs=6) as loads,
            tc.tile_pool(name="work", bufs=4) as work,
            tc.tile_pool(name="sq", bufs=2) as sqp,
            tc.tile_pool(name="acc", bufs=1) as accp,
            tc.tile_pool(name="psum", bufs=2, space="PSUM") as psum,
        ):
            mblk_t = consts.tile([128, 64], bf16)
            nc.scalar.dma_start(mblk_t[:], mblk_d.ap())
            svec_t = consts.tile([128, 1], f32)
            nc.scalar.dma_start(svec_t[:], svec_d.ap())
            acc = accp.tile([128, NPAIR], f32)

            for p in range(NPAIR):
                # 4-bank PSUM tile per chunk pair; chunk 2p in cols 0:1024,
                # chunk 2p+1 in cols 1024:2048
                ps = psum.tile([128, 2 * FREE], f32)
                for half in range(2):
                    c = 2 * p + half
                    pair = loads.tile([128, 2 * FREE], bf16)
                    nc.sync.dma_start(pair[:], big_v[c])

                    d_t = work.tile([128, FREE], bf16)
                    if c % 2 == 1:
                        nc.gpsimd.tensor_sub(d_t[:], pair[:, :FREE],
                                             pair[:, FREE:])
                    else:
                        nc.vector.tensor_sub(d_t[:], pair[:, :FREE],
                                             pair[:, FREE:])
                    dt_t = work.tile([128, FREE], bf16)
                    nc.vector.transpose(dt_t[:], d_t[:])

                    off = half * FREE
                    for j in range(FREE // 128):
                        nc.tensor.matmul(
                            ps[:, off + j * 64:off + (j + 1) * 64],
                            d_t[:, j * 128:(j + 1) * 128],
                            mblk_t[:],
                            start=True, stop=True,
                        )
                    for j in range(FREE // 128):
                        nc.tensor.matmul(
                            ps[:, off + 512 + j * 64:off + 512 + (j + 1) * 64],
                            dt_t[:, j * 128:(j + 1) * 128],
                            mblk_t[:],
                            start=True, stop=True,
                        )

                # both chunks + both terms in one ACT pass:
                # accum_out = sum over free of (svec*ps)^2
                sq = sqp.tile([128, 2 * FREE], bf16)
                nc.scalar.activation(
                    sq[:], ps[:], mybir.ActivationFunctionType.Square,
                    scale=svec_t[:], accum_out=acc[:, p:p + 1],
                )

            nc.sync.dma_start(partials.ap(), acc[:])

    nc.compile()
    _cache["nc"] = nc
    return nc


def _prep_core(arr_bf, k):
    """arr_bf: full [16, 512, 32, 32] array; returns [1024, 1024]
    chunk-major layout for core k: [ch, g, w, G, h]."""
    bpc = B // NCORES
    imgs = arr_bf[k * bpc:(k + 1) * bpc].reshape(NCHUNK, 4, 32, W, H)
    return np.ascontiguousarray(
        imgs.transpose(0, 1, 3, 2, 4)).reshape(NCHUNK * 128, FREE)


def _run(new_f, old_f, trace=False, **trace_kwargs):
    import ml_dtypes
    from concourse.bass_utils import run_bass_kernel_spmd

    nc = _build()
    mblk, svec = _consts()
    mblk_bf = mblk.astype(ml_dtypes.bfloat16)
    new_bf = np.asarray(new_f, dtype=ml_dtypes.bfloat16)
    old_bf = np.asarray(old_f, dtype=ml_dtypes.bfloat16)
    in_maps = []
    for k in range(NCORES):
        bigk = np.empty((NCHUNK * 128, 2 * FREE), dtype=ml_dtypes.bfloat16)
        bigk[:, :FREE] = _prep_core(new_bf, k)
        bigk[:, FREE:] = _prep_core(old_bf, k)
        in_maps.append({
            "big": bigk,
            "mblk": mblk_bf,
            "svec": svec,
        })
    res = run_bass_kernel_spmd(nc, in_maps, list(range(NCORES)),
                               trace=trace, **trace_kwargs)
    ss = np.float64(0.0)
    for k in range(NCORES):
        ss += np.float64(res.results[k]["partials"].astype(np.float64).sum())
    out = np.float32(0.5 * (np.float32(1e-6) + np.float32(np.sqrt(np.float32(ss)))))
    return np.asarray(out, dtype=np.float32), res


def kernel(new_f, old_f):
    out, _ = _run(np.asarray(new_f), np.asarray(old_f))
    return out
